# revision 22
# baseline (speedup 1.0000x reference)
"""Circular-convolution helper kernel for Trainium2 (8 NeuronCores).

Math: out[i] = sum_b sum_t x1[b,(i-t)%D] * x2[b,t]
            = sum_j G[j, (i-j)%D]   where G = x1^T @ x2  ([D, D], K=B contraction)

Sharding: G's rows are split across the 8 cores (core c owns rows
[128c, 128c+128)).  Per core:
  1. load xin = [x1c | x2] as one [128, 128+D] tensor, split into a 2x2
     row/column grid across the two HWDGE queues so the first G chunk can
     start as soon as the first column block lands
  2. A = x1c^T @ x2 into PSUM (K=128 fp32r matmul, 4 column chunks into
     separate PSUM banks)
  3. PSUM -> SBUF casts (fp32 -> fp32r) into a [128, 128+D] staging tile
     laid out as [A[:, 896:1024] | A] so the DRAM scatter is ONE contiguous
     region per row
  4. scatter to gd[128, 1152]: flat[1152 m + p] = staged row (4.5 KiB rows)
  5. diagonal read H[m, i] = A[m, (i-m) % D] = gd_flat[128 + 1151 m + i]
  6. ones-matmul partition collapse (fp32r): part[i] = sum_m H[m, i]
Host rotates each core's partial by 128c and sums.

Everything on the PE runs in fp16 (single-pass streaming, 10-bit mantissa
— same effective precision as tf32-style fp32r but half the DMA bytes).
PSUM accumulation stays fp32.
"""

import numpy as np

B = 128
DIM = 1024
NCORES = 8
CHUNK = DIM // NCORES  # 128 rows of G per core
NHALF = 512
NCHUNKS = 4
CW = DIM // NCHUNKS  # 256
XW = DIM + CHUNK  # packed input width: x1c | x2
XSPLIT = CHUNK + NHALF  # first column block: x1c + x2[:, 0:512]
AW = CHUNK + DIM  # staging width: wrap tail | A

USE_FP16 = True


_cached = {}


def _build():
    key = ("nc", USE_FP16)
    if key in _cached:
        return _cached[key]

    import concourse.bass as bass
    import concourse.mybir as mybir
    from concourse import bacc
    from concourse.tile import TileContext

    f32 = mybir.dt.float32
    dt_mm = mybir.dt.float16 if USE_FP16 else f32

    nc = bacc.Bacc("TRN2", target_bir_lowering=False, debug=False)

    xin = nc.dram_tensor("xin", [B, XW], dt_mm, kind="ExternalInput")
    out = nc.dram_tensor("out", [1, DIM], f32, kind="ExternalOutput")
    gd = nc.dram_tensor("gd", [CHUNK, AW], dt_mm, kind="Internal")

    with TileContext(nc) as tc:
        with (
            tc.tile_pool(name="sb", bufs=1) as sb,
            tc.tile_pool(name="ps", bufs=1, space="PSUM") as ps,
        ):
            xt = sb.tile([B, XW], dt_mm)
            xin_ap = xin.ap()
            nc.sync.dma_start(xt[0:64, 0:XSPLIT], xin_ap[0:64, 0:XSPLIT])
            nc.scalar.dma_start(xt[64:B, 0:XSPLIT], xin_ap[64:B, 0:XSPLIT])
            nc.sync.dma_start(xt[0:64, XSPLIT:XW], xin_ap[0:64, XSPLIT:XW])
            nc.scalar.dma_start(xt[64:B, XSPLIT:XW], xin_ap[64:B, XSPLIT:XW])
            x1_mm = xt[:, 0:CHUNK]

            gs = [
                ps.tile([CHUNK, CW], f32, name=f"g{i}", tag=f"g{i}")
                for i in range(NCHUNKS)
            ]
            a = sb.tile([CHUNK, AW], dt_mm)
            ht = sb.tile([CHUNK, DIM], dt_mm)
            ones = sb.tile([CHUNK, 1], dt_mm)
            nc.vector.memset(ones[:], 1.0)
            os_ = [
                ps.tile([1, CW], f32, name=f"o{i}", tag=f"o{i}")
                for i in range(NCHUNKS)
            ]
            gd_ap = gd.ap()

            # A chunks; staging layout: a[:, 0:128] = A[:, 896:1024] (wrap
            # tail), a[:, 128:1152] = A[:, 0:1024]
            order = [0, 1, 2, 3]
            for i, ch in enumerate(order):
                lo, hi = ch * CW, (ch + 1) * CW
                nc.tensor.matmul(
                    gs[ch][:], x1_mm, xt[:, CHUNK + lo : CHUNK + hi],
                    start=True, stop=True,
                )
                # alternate cast engine so the cast chain is half as long
                if i % 2 == 0:
                    nc.scalar.copy(a[:, CHUNK + lo : CHUNK + hi], gs[ch][:])
                else:
                    nc.vector.tensor_copy(a[:, CHUNK + lo : CHUNK + hi], gs[ch][:])
                if ch == 3:
                    # wrap tail: A cols [896, 1024) = chunk 3 cols [128, 256)
                    nc.vector.tensor_copy(a[:, 0:CHUNK], gs[ch][:, CHUNK:CW])

            # scatter (x2) + diagonal read (x4), chained per row block so
            # reads stream right behind the writes.
            # H[m, i] = gd_flat[128 + 1151 m + i]
            nc.sync.dma_start(gd_ap[0:64, :], a[0:64, :])
            nc.scalar.dma_start(gd_ap[64:CHUNK, :], a[64:CHUNK, :])
            for q in range(4):
                r0, r1 = q * 32, (q + 1) * 32
                diag = bass.AP(
                    gd, CHUNK + r0 * (AW - 1), [[AW - 1, 32], [1, DIM]]
                )
                r_eng = nc.sync if q % 2 == 0 else nc.scalar
                r_eng.dma_start(ht[r0:r1, :], diag)

            # ones-matmul split over K (row halves) so the first half runs
            # as soon as the first diagonal read lands
            ot = sb.tile([1, DIM], f32)
            for ch in range(NCHUNKS):
                lo, hi = ch * CW, (ch + 1) * CW
                nc.tensor.matmul(
                    os_[ch][:], ones[0:64], ht[0:64, lo:hi],
                    start=True, stop=False,
                )
            for ch in range(NCHUNKS):
                lo, hi = ch * CW, (ch + 1) * CW
                nc.tensor.matmul(
                    os_[ch][:], ones[64:CHUNK], ht[64:CHUNK, lo:hi],
                    start=False, stop=True,
                )
                if ch % 2 == 0:
                    nc.scalar.copy(ot[:, lo:hi], os_[ch][:])
                else:
                    nc.vector.tensor_copy(ot[:, lo:hi], os_[ch][:])
                if ch == 1:
                    nc.sync.dma_start(out.ap()[:, 0:NHALF], ot[:, 0:NHALF])
            nc.sync.dma_start(out.ap()[:, NHALF:DIM], ot[:, NHALF:DIM])

    nc.compile()
    _cached[key] = nc
    return nc


def _in_maps(input1, input2):
    dt_in = np.float16 if USE_FP16 else np.float32
    x1 = np.asarray(input1, dtype=np.float32)
    x2 = np.asarray(input2, dtype=np.float32)
    maps = []
    for c in range(NCORES):
        xin = np.empty((B, XW), dt_in)
        xin[:, 0:CHUNK] = x1[:, c * CHUNK : (c + 1) * CHUNK]
        xin[:, CHUNK:XW] = x2
        maps.append({"xin": np.ascontiguousarray(xin)})
    return maps


def _combine(results):
    total = np.zeros(DIM, np.float64)
    for c in range(NCORES):
        total += np.roll(results[c]["out"][0].astype(np.float64), CHUNK * c)
    return total.astype(np.float32).reshape(1, 1, DIM)


def _run(input1, input2, **kwargs):
    from concourse import bass_utils

    nc = _build()
    res = bass_utils.run_bass_kernel_spmd(
        nc, _in_maps(input1, input2), core_ids=list(range(NCORES)), **kwargs
    )
    return res


def kernel(input1, input2):
    res = _run(input1, input2)
    return _combine(res.results)


# revision 23
# speedup vs baseline: 1.0632x; 1.0632x over previous
"""Circular-convolution helper kernel for Trainium2 (8 NeuronCores).

Math: out[i] = sum_b sum_t x1[b,(i-t)%D] * x2[b,t]
            = sum_j G[j, (i-j)%D]   where G = x1^T @ x2  ([D, D], K=B contraction)

Sharding: G's rows are split across the 8 cores (core c owns rows
[128c, 128c+128)).  Per core:
  1. load xin = [x1c | x2] as one [128, 128+D] tensor, split into a 2x2
     row/column grid across the two HWDGE queues so the first G chunk can
     start as soon as the first column block lands
  2. A = x1c^T @ x2 into PSUM (K=128 fp32r matmul, 4 column chunks into
     separate PSUM banks)
  3. PSUM -> SBUF casts (fp32 -> fp32r) into a [128, 128+D] staging tile
     laid out as [A[:, 896:1024] | A] so the DRAM scatter is ONE contiguous
     region per row
  4. scatter to gd[128, 1152]: flat[1152 m + p] = staged row (4.5 KiB rows)
  5. diagonal read H[m, i] = A[m, (i-m) % D] = gd_flat[128 + 1151 m + i]
  6. ones-matmul partition collapse (fp32r): part[i] = sum_m H[m, i]
Host rotates each core's partial by 128c and sums.

Everything on the PE runs in fp16 (single-pass streaming, 10-bit mantissa
— same effective precision as tf32-style fp32r but half the DMA bytes).
PSUM accumulation stays fp32.
"""

import numpy as np

B = 128
DIM = 1024
NCORES = 8
CHUNK = DIM // NCORES  # 128 rows of G per core
NHALF = 512
NCHUNKS = 4
CW = DIM // NCHUNKS  # 256
XW = DIM + CHUNK  # packed input width: x1c | x2
XSPLIT = CHUNK + NHALF  # first column block: x1c + x2[:, 0:512]
AW = CHUNK + DIM  # staging width: wrap tail | A

USE_FP16 = True


_cached = {}


def _build():
    key = ("nc", USE_FP16)
    if key in _cached:
        return _cached[key]

    import concourse.bass as bass
    import concourse.mybir as mybir
    from concourse import bacc
    from concourse.tile import TileContext

    f32 = mybir.dt.float32
    dt_mm = mybir.dt.float16 if USE_FP16 else f32

    nc = bacc.Bacc("TRN2", target_bir_lowering=False, debug=False)

    xin = nc.dram_tensor("xin", [B, XW], dt_mm, kind="ExternalInput")
    out = nc.dram_tensor("out", [1, DIM], f32, kind="ExternalOutput")
    gd = nc.dram_tensor("gd", [CHUNK, AW], dt_mm, kind="Internal")

    with TileContext(nc) as tc:
        with (
            tc.tile_pool(name="sb", bufs=1) as sb,
            tc.tile_pool(name="ps", bufs=1, space="PSUM") as ps,
        ):
            xt = sb.tile([B, XW], dt_mm)
            xin_ap = xin.ap()
            nc.sync.dma_start(xt[0:64, 0:XSPLIT], xin_ap[0:64, 0:XSPLIT])
            nc.scalar.dma_start(xt[64:B, 0:XSPLIT], xin_ap[64:B, 0:XSPLIT])
            nc.sync.dma_start(xt[0:64, XSPLIT:XW], xin_ap[0:64, XSPLIT:XW])
            nc.scalar.dma_start(xt[64:B, XSPLIT:XW], xin_ap[64:B, XSPLIT:XW])
            x1_mm = xt[:, 0:CHUNK]

            gs = [
                ps.tile([CHUNK, CW], f32, name=f"g{i}", tag=f"g{i}")
                for i in range(NCHUNKS)
            ]
            a = sb.tile([CHUNK, AW], dt_mm)
            ht = sb.tile([CHUNK, DIM], dt_mm)
            ones = sb.tile([CHUNK, 1], dt_mm)
            nc.vector.memset(ones[:], 1.0)
            os_ = [
                ps.tile([1, CW], f32, name=f"o{i}", tag=f"o{i}")
                for i in range(NCHUNKS)
            ]
            gd_ap = gd.ap()

            # A chunks; staging layout: a[:, 0:128] = A[:, 896:1024] (wrap
            # tail), a[:, 128:1152] = A[:, 0:1024]
            order = [0, 1, 2, 3]
            for i, ch in enumerate(order):
                lo, hi = ch * CW, (ch + 1) * CW
                nc.tensor.matmul(
                    gs[ch][:], x1_mm, xt[:, CHUNK + lo : CHUNK + hi],
                    start=True, stop=True,
                )
                # alternate cast engine so the cast chain is half as long
                if i % 2 == 0:
                    nc.scalar.copy(a[:, CHUNK + lo : CHUNK + hi], gs[ch][:])
                else:
                    nc.vector.tensor_copy(a[:, CHUNK + lo : CHUNK + hi], gs[ch][:])
                if ch == 3:
                    # wrap tail: A cols [896, 1024) = chunk 3 cols [128, 256)
                    nc.vector.tensor_copy(a[:, 0:CHUNK], gs[ch][:, CHUNK:CW])

            # scatter + diagonal read, chained in 4 row blocks so reads
            # stream right behind the writes.
            # H[m, i] = gd_flat[128 + 1151 m + i]
            for q in range(4):
                r0, r1 = q * 32, (q + 1) * 32
                w_eng = nc.sync if q % 2 == 0 else nc.scalar
                w_eng.dma_start(gd_ap[r0:r1, :], a[r0:r1, :])
            for q in range(4):
                r0, r1 = q * 32, (q + 1) * 32
                diag = bass.AP(
                    gd, CHUNK + r0 * (AW - 1), [[AW - 1, 32], [1, DIM]]
                )
                r_eng = nc.sync if q % 2 == 0 else nc.scalar
                r_eng.dma_start(ht[r0:r1, :], diag)

            # ones-matmul split over K (row halves) so the first half runs
            # as soon as the first diagonal read lands
            ot = sb.tile([1, DIM], f32)
            for ch in range(NCHUNKS):
                lo, hi = ch * CW, (ch + 1) * CW
                nc.tensor.matmul(
                    os_[ch][:], ones[0:64], ht[0:64, lo:hi],
                    start=True, stop=False,
                )
            for ch in range(NCHUNKS):
                lo, hi = ch * CW, (ch + 1) * CW
                nc.tensor.matmul(
                    os_[ch][:], ones[64:CHUNK], ht[64:CHUNK, lo:hi],
                    start=False, stop=True,
                )
                if ch % 2 == 0:
                    nc.scalar.copy(ot[:, lo:hi], os_[ch][:])
                else:
                    nc.vector.tensor_copy(ot[:, lo:hi], os_[ch][:])
            nc.sync.dma_start(out.ap(), ot[:])

    nc.compile()
    _cached[key] = nc
    return nc


def _in_maps(input1, input2):
    dt_in = np.float16 if USE_FP16 else np.float32
    x1 = np.asarray(input1, dtype=np.float32)
    x2 = np.asarray(input2, dtype=np.float32)
    maps = []
    for c in range(NCORES):
        xin = np.empty((B, XW), dt_in)
        xin[:, 0:CHUNK] = x1[:, c * CHUNK : (c + 1) * CHUNK]
        xin[:, CHUNK:XW] = x2
        maps.append({"xin": np.ascontiguousarray(xin)})
    return maps


def _combine(results):
    total = np.zeros(DIM, np.float64)
    for c in range(NCORES):
        total += np.roll(results[c]["out"][0].astype(np.float64), CHUNK * c)
    return total.astype(np.float32).reshape(1, 1, DIM)


def _run(input1, input2, **kwargs):
    from concourse import bass_utils

    nc = _build()
    res = bass_utils.run_bass_kernel_spmd(
        nc, _in_maps(input1, input2), core_ids=list(range(NCORES)), **kwargs
    )
    return res


def kernel(input1, input2):
    res = _run(input1, input2)
    return _combine(res.results)
